# revision 7
# baseline (speedup 1.0000x reference)
"""DGMNet (dense MLP, 4 DGM layers) Trainium2 kernel — fp8/bf16 mixed, v4.

Data-parallel over batch (65536 -> 8 cores x 8192), feature-major
activations as packed wide tiles ([128 feat x 8 m-tiles x 512 batch]).

Speed structure:

1. Algebraic collapse: S1 = x Sw.T + b is affine in the 16-dim input, so
   every matmul against S1 (wgS1, layer-0 Z, G, layer-0 R) folds into a
   K=16 matmul with host-precomputed weights.

2. Biases ride the matmul: x gets a 17th "ones" row and U a 17th bias
   row, so psum = W.x + b directly and evacuations need no per-m bias.
   This enables FUSED evacuations: one ACT instruction per 4 PSUM banks
   ([128, 2048]) instead of four — the scalar engine's ~352-cycle fixed
   overhead per instruction is a co-bottleneck otherwise.

3. The Z/R gates (layers>=1) and the H gates of layers 0-2 run in fp8e4
   DoubleRow (full-width N=512 pair instructions; weights host-quantized
   at scale 32, 1/32 descale on the tanh evacuation).  Layer 3's H runs
   bf16 (its error feeds the output directly; fp8 there blows the 2e-2
   budget — simulated 1.855e-2 with this split vs 2.21e-2 all-fp8).

4. PSUM as two 4-bank [128, 2048] tiles (ping-pong); the final output
   row accumulates into a bank-0 view of the last gate tile.

5. TWO batch tiles processed interleaved layer-by-layer: tile B's PE
   chains hide tile A's boundary glue latency (H-evac -> combine ->
   tanh->fp8 repack is a ~10us serial ACT/DVE chain per layer boundary
   that would otherwise idle the PE).  z*s is hoisted before the H gate
   to shorten that chain.
"""

import sys

sys.path.insert(0, "/opt/trn_rl_repo")

import numpy as np

B_FULL = 65536
KI = 16
KB = 17                # 16 x-features + ones row (bias fold)
H = 1024
NCORES = 8
BC = B_FULL // NCORES  # per-core batch (8192)
NB = 512               # batch tile (one PSUM bank of fp32)
NM = H // 128          # feature tiles (8)
NJ = NM // 2           # fp8 DoubleRow k-tile pairs (4)
N_LAYERS = 4
NG = 8                 # gate blocks in the packed U tensor
WS = 32.0              # fp8 weight scale

MM_DT = "float32r"

_BUILD_CACHE = {}


def _build(bc, nb, mm_dt, repeat=1):
    import concourse.bacc as bacc
    import concourse.mybir as mybir
    import concourse.tile as tile

    f32 = mybir.dt.float32
    bf16 = mybir.dt.bfloat16
    f8 = mybir.dt.float8e4
    mdt = getattr(mybir.dt, mm_dt)
    DR = mybir.MatmulPerfMode.DoubleRow
    Tanh = mybir.ActivationFunctionType.Tanh
    Ident = mybir.ActivationFunctionType.Identity
    mult = mybir.AluOpType.mult
    sub = mybir.AluOpType.subtract

    nt = bc // nb

    nc = bacc.Bacc("TRN2", target_bir_lowering=False, debug=False,
                   num_devices=NCORES)

    xT_d = nc.dram_tensor("xT", [KB, bc], mdt, kind="ExternalInput").ap()
    wpz_d = nc.dram_tensor("WPZ", [128, 2, NJ * NM * 128], f8,
                           kind="ExternalInput").ap()
    wpg_d = nc.dram_tensor("WPG", [128, 2, NJ * NM * 128], f8,
                           kind="ExternalInput").ap()
    wgb_d = nc.dram_tensor("WGB", [H, H], bf16, kind="ExternalInput").ap()
    u_d = nc.dram_tensor("U", [128, NG * H], mdt, kind="ExternalInput").ap()
    ow_d = nc.dram_tensor("OW", [128, NM], bf16, kind="ExternalInput").ap()
    ob_d = nc.dram_tensor("OB", [1, 1], f32, kind="ExternalInput").ap()
    y_d = nc.dram_tensor("Y", [1, bc], f32, kind="ExternalOutput").ap()

    with tile.TileContext(nc) as tc:
        with (
            tc.tile_pool(name="const", bufs=1) as cpool,
            tc.tile_pool(name="xt", bufs=3) as xt_pool,
            tc.tile_pool(name="act", bufs=2) as act_pool,
            tc.tile_pool(name="ov", bufs=2) as ov_pool,
            tc.tile_pool(name="psum", bufs=2, space="PSUM") as ps_pool,
        ):
            # ---- resident constants -------------------------------------
            u_sb = cpool.tile([128, NG * H], mdt)
            nc.gpsimd.dma_start(u_sb[:], u_d[:])
            ow_sb = cpool.tile([128, NM], bf16)
            nc.gpsimd.dma_start(ow_sb[:], ow_d[:])
            ob_sb = cpool.tile([1, 1], f32)
            nc.gpsimd.dma_start(ob_sb[:], ob_d[:])
            wpg_sb = cpool.tile([128, 2, NJ * NM * 128], f8)
            nc.gpsimd.dma_start(wpg_sb[:], wpg_d[:])
            wpz_sb = cpool.tile([128, 2, NJ * NM * 128], f8)
            nc.gpsimd.dma_start(wpz_sb[:], wpz_d[:])
            wgb_sb = cpool.tile([128, NM * H], bf16)
            for k in range(NM):
                nc.gpsimd.dma_start(wgb_sb[:, k * H:(k + 1) * H],
                                    wgb_d[k * 128:(k + 1) * 128, :])

            def w8_ap(w_sb, j, m):
                o = (j * NM + m) * 128
                return w_sb[:, :, o:o + 128]

            def wb_ap(k, m):
                return wgb_sb[:, k * H + m * 128:k * H + (m + 1) * 128]

            def u_ap(g, m, c):
                return u_sb[32 * c:32 * c + KB,
                            g * H + m * 128:g * H + (m + 1) * 128]

            def starts(gate, big, xt, half, single):
                """Four concurrent K=17 (weights+bias) start matmuls, one
                per 32-row PE band, each into its own bank of `big`."""
                for c in range(4):
                    m = 4 * half + c
                    nc.tensor.matmul(
                        big[:, c * nb:(c + 1) * nb], u_ap(gate, m, c),
                        xt[32 * c:32 * c + KB, :],
                        start=True, stop=single,
                        tile_position=(32 * c, 0))

            def k17_gate(gate, xt, dest, nametag, actfn):
                """Pure x-side gate: K=17 starts + fused 4-bank evac.
                actfn=None -> plain copy on the DVE (keeps ACT free)."""
                last = None
                for half in (0, 1):
                    big = ps_pool.tile([128, 4 * nb], f32, tag="big",
                                       name=f"ps_{nametag}_{half}")
                    starts(gate, big, xt, half, single=True)
                    if actfn is None:
                        nc.vector.tensor_copy(
                            dest[:, 4 * half:4 * half + 4, :], big[:])
                    else:
                        nc.scalar.activation(
                            dest[:, 4 * half:4 * half + 4, :],
                            big[:], actfn)
                    last = big
                return last

            def hh8_gate(gate, wp, mov, dest, xt, nametag):
                """fp8 DoubleRow gate: K=17 start + 4 full-width DR pair
                instrs per m; fused tanh(psum/32) evacuation."""
                last = None
                for half in (0, 1):
                    big = ps_pool.tile([128, 4 * nb], f32, tag="big",
                                       name=f"ps_{nametag}_{half}")
                    starts(gate, big, xt, half, single=False)
                    for c in range(4):
                        m = 4 * half + c
                        for j in range(NJ):
                            nc.tensor.matmul(
                                big[:, c * nb:(c + 1) * nb],
                                w8_ap(wp, j, m),
                                mov[:, 2 * j:2 * j + 2, :],
                                start=False, stop=(j == NJ - 1),
                                perf_mode=DR)
                    nc.scalar.activation(dest[:, 4 * half:4 * half + 4, :],
                                         big[:], Tanh, scale=1.0 / WS)
                    last = big
                return last

            def hhb_gate(gate, mov, dest, xt, nametag):
                """bf16 H gate (layer 3): K=17 start + K=1024 bf16 chain."""
                last = None
                for half in (0, 1):
                    big = ps_pool.tile([128, 4 * nb], f32, tag="big",
                                       name=f"ps_{nametag}_{half}")
                    starts(gate, big, xt, half, single=False)
                    for c in range(4):
                        m = 4 * half + c
                        for k in range(NM):
                            nc.tensor.matmul(
                                big[:, c * nb:(c + 1) * nb],
                                wb_ap(k, m), mov[:, k, :],
                                start=False, stop=(k == NM - 1))
                    nc.scalar.activation(dest[:, 4 * half:4 * half + 4, :],
                                         big[:], Tanh)
                    last = big
                return last

            def wide(dt_, tag, name):
                return act_pool.tile([128, NM, nb], dt_, tag=tag, name=name)

            # ---- one batch tile as a 5-stage generator ------------------
            def tile_stages(t, t_u):
                xt_cur = xt_pool.tile([128, nb], mdt, tag="xt",
                                      name=f"xt_{t_u}")
                for c in range(4):
                    nc.gpsimd.dma_start(xt_cur[32 * c:32 * c + KB, :],
                                        xT_d[:, t * nb:(t + 1) * nb])

                s_all = wide(bf16, "s", f"s_{t_u}_0")
                r_all = wide(bf16, "r", f"r_{t_u}_0")
                g_all = wide(bf16, "g", f"g_{t_u}")
                z_all = wide(bf16, "z", f"z_{t_u}_0")
                # S1 and R0 first so SR production starts early
                k17_gate(0, xt_cur, s_all, f"s1_{t_u}", None)
                k17_gate(3, xt_cur, r_all, f"r0_{t_u}", Tanh)
                k17_gate(1, xt_cur, g_all, f"g_{t_u}", Tanh)
                k17_gate(2, xt_cur, z_all, f"z0_{t_u}", Tanh)
                sr8 = wide(f8, "sr8", f"sr8_{t_u}_0")
                nc.vector.tensor_mul(sr8[:], s_all[:], r_all[:])
                yield

                s8_all = None
                last_big = None
                for i in range(N_LAYERS):
                    if i > 0:
                        r_all = wide(bf16, "r", f"r_{t_u}_{i}")
                        hh8_gate(5, wpg_sb, s8_all, r_all, xt_cur,
                                 f"r_{t_u}_{i}")
                        z_all = wide(bf16, "z", f"z_{t_u}_{i}")
                        hh8_gate(4, wpz_sb, s8_all, z_all, xt_cur,
                                 f"z_{t_u}_{i}")

                    h_all = wide(bf16, "h", f"h_{t_u}_{i}")
                    if i > 0:
                        if i < N_LAYERS - 1:
                            sr8 = wide(f8, "sr8", f"sr8_{t_u}_{i}")
                            nc.vector.tensor_mul(sr8[:], s_all[:],
                                                 r_all[:])
                        else:
                            srb = wide(bf16, "srb", f"srb_{t_u}")
                            nc.vector.tensor_mul(srb[:], s_all[:],
                                                 r_all[:])
                    # z*s hoisted off the boundary critical path
                    nc.vector.tensor_mul(z_all[:], z_all[:], s_all[:])
                    if i < N_LAYERS - 1:
                        last_big = hh8_gate(6, wpg_sb, sr8, h_all, xt_cur,
                                            f"h_{t_u}_{i}")
                    else:
                        last_big = hhb_gate(7, srb, h_all, xt_cur,
                                            f"h_{t_u}_{i}")

                    # output = (1-G)*H + Z*S  as  (z*s) - (g-1)*h
                    nc.vector.scalar_tensor_tensor(
                        h_all[:], g_all[:], 1.0, h_all[:],
                        op0=sub, op1=mult)
                    nc.vector.tensor_sub(h_all[:], z_all[:], h_all[:])

                    if i < N_LAYERS - 1:
                        # tanh once on ACT; the fp8 DR repack is a DVE
                        # copy so ACT (the #2 engine) stays off the
                        # critical path
                        s_all = wide(bf16, "s", f"s_{t_u}_{i + 1}")
                        nc.scalar.activation(s_all[:], h_all[:], Tanh)
                        s8_all = wide(f8, "s8", f"s8_{t_u}_{i + 1}")
                        nc.vector.tensor_copy(s8_all[:], s_all[:])
                    if i < N_LAYERS - 1:
                        yield

                # out row accumulates into a bank-0 view of the last
                # gate's second PSUM tile (clears only that bank's
                # has_written bits; its data is already evacuated)
                po = last_big[0:1, 0:nb]
                for k in range(NM):
                    nc.tensor.matmul(po, ow_sb[:, k:k + 1],
                                     h_all[:, k, :],
                                     start=(k == 0), stop=(k == NM - 1))
                orow = ov_pool.tile([1, nb], f32, tag="orow",
                                    name=f"orow_{t_u}")
                nc.vector.tensor_scalar_add(orow[:], po, ob_sb[0:1, 0:1])
                nc.gpsimd.dma_start(y_d[0:1, t * nb:(t + 1) * nb],
                                    orow[:])
                yield

            # ---- drive pairs of tiles interleaved stage-by-stage --------
            total = repeat * nt
            for p in range(total // 2):
                ta, tb = 2 * p, 2 * p + 1
                ga = tile_stages(ta % nt, ta)
                gb = tile_stages(tb % nt, tb)
                for _ in range(N_LAYERS + 1):
                    next(ga, None)
                    next(gb, None)

    nc.compile()
    return nc


def _get_nc(bc=BC, nb=NB, mm_dt=MM_DT):
    key = (bc, nb, mm_dt)
    if key not in _BUILD_CACHE:
        _BUILD_CACHE[key] = _build(bc, nb, mm_dt)
    return _BUILD_CACHE[key]


def _pack_w8(WT):
    """[H, H] f32 -> [128, 2, NJ*NM*128] fp8 DoubleRow slot layout.
    WT rows = input features (contraction), cols = output features."""
    import ml_dtypes
    F8 = ml_dtypes.float8_e4m3
    W32 = np.asarray(WT, np.float32) * np.float32(WS)
    hi = W32.astype(F8)
    # [1024, 1024] -> [j(4), i(2), p(128), m(8), mm(128)] -> [p, i, j*m, mm]
    A = np.asarray(hi).reshape(NJ, 2, 128, NM, 128)
    A = A.transpose(2, 1, 0, 3, 4).reshape(128, 2, NJ * NM * 128)
    return np.ascontiguousarray(A)


def _prep_inputs(x, Sw_w, Sw_b, Uz_w, Uz_b, Wz_w, Wz_b, Ug_w, Ug_b, Wg_w,
                 Wg_b, Ur_w, Ur_b, Uh_w, Uh_b, out_w, out_b):
    import ml_dtypes
    f = np.float32
    d = np.float64
    B = np.asarray(x, f).shape[0]
    xT = np.empty((KB, B), f)
    xT[:KI] = np.asarray(x, f).T
    xT[KI] = 1.0
    WzT = np.ascontiguousarray(np.asarray(Wz_w, f).T)           # [H, H]
    WgT = np.ascontiguousarray(np.asarray(Wg_w, f).T)
    WPZ = _pack_w8(WzT)
    WPG = _pack_w8(WgT)
    WGB = np.ascontiguousarray(WgT.astype(ml_dtypes.bfloat16))
    WzSw = np.asarray(Wz_w, d) @ np.asarray(Sw_w, d)            # [H, 16]
    WgSw = np.asarray(Wg_w, d) @ np.asarray(Sw_w, d)
    WzSb = np.asarray(Wz_w, d) @ np.asarray(Sw_b, d)            # [H]
    WgSb = np.asarray(Wg_w, d) @ np.asarray(Sw_b, d)
    Ug_eff = (np.asarray(Ug_w, d) + WgSw).astype(f)             # G
    Uz_eff = (np.asarray(Uz_w, d) + WzSw).astype(f)             # layer-0 Z
    Ur_eff = (np.asarray(Ur_w, d) + WgSw).astype(f)             # layer-0 R
    U16 = np.concatenate(
        [np.asarray(Sw_w, f).T, Ug_eff.T, Uz_eff.T, Ur_eff.T,
         np.asarray(Uz_w, f).T * f(WS), np.asarray(Ur_w, f).T * f(WS),
         np.asarray(Uh_w, f).T * f(WS), np.asarray(Uh_w, f).T],
        axis=1)                                                 # [16, NG*H]
    biases = [
        np.asarray(Sw_b, d),
        np.asarray(Ug_b, d) + np.asarray(Wg_b, d) + WgSb,
        np.asarray(Uz_b, d) + np.asarray(Wz_b, d) + WzSb,
        np.asarray(Ur_b, d) + np.asarray(Wg_b, d) + WgSb,
        (np.asarray(Uz_b, d) + np.asarray(Wz_b, d)) * WS,
        (np.asarray(Ur_b, d) + np.asarray(Wg_b, d)) * WS,
        (np.asarray(Uh_b, d) + np.asarray(Wg_b, d)) * WS,
        np.asarray(Uh_b, d) + np.asarray(Wg_b, d),
    ]
    B16 = np.concatenate([b.astype(f) for b in biases])         # [NG*H]
    U = np.zeros((128, NG * H), f)
    for c in range(4):
        U[32 * c:32 * c + KI] = U16
        U[32 * c + KI] = B16
    OW = np.ascontiguousarray(
        np.asarray(out_w, f).reshape(NM, 128).T.astype(ml_dtypes.bfloat16))
    OB = np.asarray(out_b, f).reshape(1, 1)
    return xT, WPZ, WPG, WGB, U, OW, OB


def kernel(**inputs):
    from concourse.bass_utils import run_bass_kernel_spmd

    nc = _get_nc()
    in_maps = _make_in_maps(inputs)
    res = run_bass_kernel_spmd(nc, in_maps, list(range(NCORES)))
    y = np.concatenate([res.results[c]["Y"] for c in range(NCORES)], axis=1)
    return np.ascontiguousarray(y.reshape(B_FULL, 1)).astype(np.float32)


def _make_in_maps(inputs):
    xT, WPZ, WPG, WGB, U, OW, OB = _prep_inputs(**inputs)
    return [{
        "xT": np.ascontiguousarray(xT[:, c * BC:(c + 1) * BC]),
        "WPZ": WPZ, "WPG": WPG, "WGB": WGB,
        "U": U, "OW": OW, "OB": OB,
    } for c in range(NCORES)]


def timed_run(inputs, iters=5, nc=None, pipeline=1):
    import time
    import jax
    from jax.sharding import Mesh, PartitionSpec, NamedSharding
    from jax.experimental.shard_map import shard_map
    from concourse import bass2jax, mybir

    bass2jax.install_neuronx_cc_hook()
    if nc is None:
        nc = _get_nc()
    in_maps = _make_in_maps(inputs)
    n_cores = NCORES

    partition_name = (nc.partition_id_tensor.name
                      if nc.partition_id_tensor else None)
    in_names, out_names, out_avals, zero_outs = [], [], [], []
    for alloc in nc.m.functions[0].allocations:
        if not isinstance(alloc, mybir.MemoryLocationSet):
            continue
        name = alloc.memorylocations[0].name
        if alloc.kind == "ExternalInput":
            if name != partition_name:
                in_names.append(name)
        elif alloc.kind == "ExternalOutput":
            shape = tuple(alloc.tensor_shape)
            dtype = mybir.dt.np(alloc.dtype)
            out_names.append(name)
            out_avals.append(jax.core.ShapedArray(shape, dtype))
            zero_outs.append(np.zeros(shape, dtype))
    n_params = len(in_names)
    n_outs = len(out_avals)
    all_in = list(in_names) + list(out_names)
    if partition_name is not None:
        all_in.append(partition_name)
    donate = tuple(range(n_params, n_params + n_outs))

    def _body(*args):
        operands = list(args)
        if partition_name is not None:
            operands.append(bass2jax.partition_id_tensor())
        outs = bass2jax._bass_exec_p.bind(
            *operands,
            out_avals=tuple(out_avals),
            in_names=tuple(all_in),
            out_names=tuple(out_names),
            lowering_input_output_aliases=(),
            sim_require_finite=False,
            sim_require_nnan=False,
            nc=nc,
        )
        return tuple(outs)

    devices = jax.devices()[:n_cores]
    mesh = Mesh(np.asarray(devices), ("core",))
    spec = PartitionSpec("core")
    sharded = jax.jit(
        shard_map(_body, mesh=mesh, in_specs=(spec,) * (n_params + n_outs),
                  out_specs=(spec,) * n_outs, check_rep=False),
        donate_argnums=donate, keep_unused=True)

    sharding = NamedSharding(mesh, spec)
    dev_in = [
        jax.device_put(
            np.concatenate([np.asarray(in_maps[c][n]) for c in range(n_cores)],
                           axis=0), sharding)
        for n in in_names
    ]
    def fresh_zeros():
        return [np.zeros((n_cores * z.shape[0], *z.shape[1:]), z.dtype)
                for z in zero_outs]

    outs = sharded(*dev_in, *fresh_zeros())
    jax.block_until_ready(outs)

    state = {"outs": outs}

    def run_once(pipeline_n=pipeline):
        zss = [fresh_zeros() for _ in range(pipeline_n)]
        t0 = time.perf_counter()
        all_outs = [sharded(*dev_in, *zs) for zs in zss]
        jax.block_until_ready(all_outs)
        state["outs"] = all_outs[-1]
        return int((time.perf_counter() - t0) * 1e9 / pipeline_n)

    def get_y():
        y = np.asarray(state["outs"][out_names.index("Y")])  # [8, BC]
        return np.ascontiguousarray(
            y.reshape(1, B_FULL).reshape(B_FULL, 1)).astype(np.float32)

    if iters is None:
        return run_once, get_y

    times = [run_once() for _ in range(iters)]
    return min(times), times, get_y()
